# revision 3
# baseline (speedup 1.0000x reference)
"""Trainium2 Bass kernel for nn_CausalSE (chunked-EMA squeeze-excite gating).

Reference computation (per batch b):
    xc   = mean over chunks of 16 along L            -> [C, N]   (N = L/16)
    e_t  = g*e_{t-1} + (1-g)*xc_t   (causal EMA)     -> [C, N]
    h    = relu(w1 @ e + b1)                         -> [C/8, N]
    gate = sigmoid(w2 @ h + b2)                      -> [C, N]
    out  = repeat(gate, 16) * x                      -> [C, L]

Distribution: pure data-parallel over batch. B == 8 == n_cores, each core
processes one full batch element independently; no collectives.

Math transform used on-chip: let u_t = g*u_{t-1} + sum16(x)_t (plain scan on
pooled *sums*).  Then e = ((1-g)/16) * u, which is folded into w1 on the host
(w1s = w1 * ((1-g)/16)).  This removes a per-element rescale pass on DVE.
"""

import numpy as np
from contextlib import ExitStack

import concourse.bass as bass
import concourse.tile as tile
from concourse import bacc, mybir

F32 = mybir.dt.float32
P = 128


def build_graph(C=512, L=8192, CS=16, HID=64, NL=4, reps=1):
    """Build the per-core Bass graph (SPMD: every core runs this same graph).

    NL: number of column chunks the L axis is split into for pipelining.
    reps: repeat the whole computation (for on-device timing via slope).
    """
    NCT = C // P          # channel partition-tiles
    LC = L // NL          # x columns per chunk
    NCc = LC // CS        # pooled columns per chunk

    nc = bacc.Bacc(None, target_bir_lowering=False)

    x_ext = nc.declare_dram_parameter("x", [C, L], F32, isOutput=False)
    w1_ext = nc.declare_dram_parameter("w1s", [P, NCT * HID], F32, isOutput=False)
    w2_ext = nc.declare_dram_parameter("w2t", [HID, C], F32, isOutput=False)
    b1_ext = nc.declare_dram_parameter("b1", [HID, 1], F32, isOutput=False)
    b2_ext = nc.declare_dram_parameter("b2", [P, NCT], F32, isOutput=False)
    g_ext = nc.declare_dram_parameter("g", [P, NCT], F32, isOutput=False)
    out_ext = nc.declare_dram_parameter("out", [C, L], F32, isOutput=True)

    with ExitStack() as ctx:
        tc = ctx.enter_context(tile.TileContext(nc))
        consts = ctx.enter_context(tc.tile_pool(name="consts", bufs=1))
        xpool = ctx.enter_context(tc.tile_pool(name="xpool", bufs=2))
        small = ctx.enter_context(tc.tile_pool(name="small", bufs=2))
        psum = ctx.enter_context(
            tc.tile_pool(name="psum", bufs=2, space=bass.MemorySpace.PSUM)
        )

        w1_sb = consts.tile([P, NCT, HID], F32)
        nc.gpsimd.dma_start(
            out=w1_sb[:], in_=w1_ext[:].rearrange("p (ct h) -> p ct h", ct=NCT)
        )
        w2_sb = consts.tile([HID, C], F32)
        nc.gpsimd.dma_start(out=w2_sb[:], in_=w2_ext[:])
        b1_sb = consts.tile([HID, 1], F32)
        nc.gpsimd.dma_start(out=b1_sb[:], in_=b1_ext[:])
        b2_sb = consts.tile([P, NCT], F32)
        nc.gpsimd.dma_start(out=b2_sb[:], in_=b2_ext[:])
        g_sb = consts.tile([P, NCT], F32)
        nc.gpsimd.dma_start(out=g_sb[:], in_=g_ext[:])

        # broadcast gamma along the free axis for the scan's data0 operand
        ones = consts.tile([P, NCc], F32)
        nc.vector.memset(ones[:], 1.0)
        g_bcast = []
        for ct in range(NCT):
            gb = consts.tile([P, NCc], F32, tag=f"gb{ct}")
            nc.vector.tensor_scalar_mul(gb[:], ones[:], g_sb[:, ct : ct + 1])
            g_bcast.append(gb)

        for _r in range(reps):
            u_prev = [None] * NCT
            for k in range(NL):
                xts = []
                uts = []
                for ct in range(NCT):
                    x_t = xpool.tile([P, LC], F32, tag=f"x{ct}")
                    nc.sync.dma_start(
                        out=x_t[:],
                        in_=x_ext[ct * P : (ct + 1) * P, k * LC : (k + 1) * LC],
                    )
                    xc_t = small.tile([P, NCc], F32, tag=f"xc{ct}")
                    nc.vector.tensor_reduce(
                        out=xc_t[:],
                        in_=x_t[:].rearrange("p (n j) -> p n j", j=CS),
                        axis=mybir.AxisListType.X,
                        op=mybir.AluOpType.add,
                    )
                    u_t = small.tile([P, NCc], F32, tag=f"u{ct}")
                    init = 0.0 if k == 0 else u_prev[ct][:, NCc - 1 : NCc]
                    nc.vector.tensor_tensor_scan(
                        out=u_t[:],
                        data0=g_bcast[ct][:],
                        data1=xc_t[:],
                        initial=init,
                        op0=mybir.AluOpType.mult,
                        op1=mybir.AluOpType.add,
                    )
                    xts.append(x_t)
                    uts.append(u_t)

                # SE bottleneck: h = relu(w1s @ u + b1)
                h_ps = psum.tile([HID, NCc], F32, tag="hps")
                for ct in range(NCT):
                    nc.tensor.matmul(
                        h_ps[:],
                        w1_sb[:, ct, :],
                        uts[ct][:],
                        start=(ct == 0),
                        stop=(ct == NCT - 1),
                    )
                h_sb = small.tile([HID, NCc], F32, tag="h")
                nc.scalar.activation(
                    out=h_sb[:],
                    in_=h_ps[:],
                    func=mybir.ActivationFunctionType.Relu,
                    bias=b1_sb[:],
                )
                for ct in range(NCT):
                    o_ps = psum.tile([P, NCc], F32, tag="ops")
                    nc.tensor.matmul(
                        o_ps[:],
                        w2_sb[:, ct * P : (ct + 1) * P],
                        h_sb[:],
                        start=True,
                        stop=True,
                    )
                    gate_t = small.tile([P, NCc], F32, tag="gate")
                    nc.scalar.activation(
                        out=gate_t[:],
                        in_=o_ps[:],
                        func=mybir.ActivationFunctionType.Sigmoid,
                        bias=b2_sb[:, ct : ct + 1],
                    )
                    x3 = xts[ct][:].rearrange("p (n j) -> p n j", j=CS)
                    g_ap = gate_t[:]
                    gate_b = bass.AP(
                        tensor=g_ap.tensor,
                        offset=g_ap.offset,
                        ap=[list(g_ap.ap[0]), list(g_ap.ap[1]), [0, CS]],
                    )
                    nc.vector.tensor_tensor(
                        out=x3, in0=x3, in1=gate_b, op=mybir.AluOpType.mult
                    )
                    nc.scalar.dma_start(
                        out=out_ext[ct * P : (ct + 1) * P, k * LC : (k + 1) * LC],
                        in_=xts[ct][:],
                    )
                u_prev = uts

    nc.compile()
    return nc


def host_prep(gamma, w1, b1, w2, b2, C=512, HID=64):
    """Host-side preprocessing of the shared (small) tensors."""
    NCT = C // P
    gamma = np.asarray(gamma, np.float32)
    w1 = np.asarray(w1, np.float32)
    w2 = np.asarray(w2, np.float32)
    bv = (1.0 - gamma) / 16.0
    w1s = (w1 * bv[None, :]).T  # [C, HID]
    # [C, HID] -> [P, NCT*HID] with c = ct*P + p
    w1s_r = np.ascontiguousarray(
        w1s.reshape(NCT, P, HID).transpose(1, 0, 2).reshape(P, NCT * HID)
    )
    w2t = np.ascontiguousarray(w2.T)  # [HID, C]
    b1_r = np.ascontiguousarray(np.asarray(b1, np.float32).reshape(HID, 1))
    b2_r = np.ascontiguousarray(np.asarray(b2, np.float32).reshape(NCT, P).T)
    g_r = np.ascontiguousarray(gamma.reshape(NCT, P).T)
    return w1s_r, w2t, b1_r, b2_r, g_r


DEFAULT_NL = 4

_GRAPH_CACHE = {}


def _get_graph(key=(512, 8192, 16, 64, 4, 1)):
    if key not in _GRAPH_CACHE:
        _GRAPH_CACHE[key] = build_graph(*key)
    return _GRAPH_CACHE[key]


def make_in_maps(x, gamma, w1, b1, w2, b2):
    B, C, L = x.shape
    HID = w1.shape[0]
    w1s_r, w2t, b1_r, b2_r, g_r = host_prep(gamma, w1, b1, w2, b2, C=C, HID=HID)
    x = np.asarray(x, np.float32)
    return [
        {
            "x": np.ascontiguousarray(x[b]),
            "w1s": w1s_r,
            "w2t": w2t,
            "b1": b1_r,
            "b2": b2_r,
            "g": g_r,
        }
        for b in range(B)
    ]


def kernel(x, gamma, w1, b1, w2, b2):
    from concourse.bass_utils import run_bass_kernel_spmd

    B, C, L = x.shape
    nc = _get_graph((C, L, 16, w1.shape[0], DEFAULT_NL, 1))
    in_maps = make_in_maps(x, gamma, w1, b1, w2, b2)
    res = run_bass_kernel_spmd(nc, in_maps, core_ids=list(range(B)))
    out = np.stack([res.results[b]["out"] for b in range(B)], axis=0)
    return out.astype(np.float32)
